# revision 7
# baseline (speedup 1.0000x reference)
"""Trainium2 Bass kernel for CustomMHA (B=4, S=2048, D=1024, H=16, rope, causal).

Sharding: 8 cores = 4 batches x 2 head-groups (8 heads each).
Per core: QKV projections (f32r matmuls), RoPE fused via DVE + PE transpose-
accumulate, attention computed as S^T = K^T.T @ Q^T tiles (exp on ACT ->
P^T directly in SBUF, no P transpose needed), AV with a ones-row appended to V
so softmax denominators fall out of the same matmul, per-head normalization via
PE partition-broadcast + DVE fast reciprocal, then the output projection of the
local 512 ctx dims. Host sums the two head-group partials per batch.
"""

import numpy as np

import concourse.bass as bass
import concourse.tile as tile
from concourse import bacc, masks, mybir
from concourse.bass_utils import run_bass_kernel_spmd

B, S, D, H = 4, 2048, 1024, 16
HD = 64          # head dim
P = 128          # partitions
NT = S // P      # 16 s-tiles
HL = H // 2      # 8 heads per core
GL = HL // 2     # 4 head-pair groups per core
DL = HL * HD     # 512 local ctx dims
KCH = D // P     # 8 contraction chunks
NSQ = 4          # sq chunks
SQW = S // NSQ   # 512
F32 = mybir.dt.float32
F32R = mybir.dt.float32r


def _r(ap):
    return ap.bitcast(F32R)


def _build(causal: bool, has_bq: bool, has_bk: bool, has_bv: bool):
    nc = bacc.Bacc("TRN2", target_bir_lowering=False, debug=False, num_devices=8)

    xtp = nc.declare_dram_parameter("xtp", [NT, P, KCH, P], F32, isOutput=False)
    wq = nc.declare_dram_parameter("wq", [P, KCH, DL], F32, isOutput=False)
    wk = nc.declare_dram_parameter("wk", [P, KCH, DL], F32, isOutput=False)
    wv = nc.declare_dram_parameter("wv", [P, KCH, DL], F32, isOutput=False)
    wo = nc.declare_dram_parameter("wo", [P, GL, D], F32, isOutput=False)
    cos_d = nc.declare_dram_parameter("cos", [NT, P, HD], F32, isOutput=False)
    ss_d = nc.declare_dram_parameter("ss", [NT, P, HD], F32, isOutput=False)
    ident_d = nc.declare_dram_parameter("ident", [P, P], F32, isOutput=False)
    ones_d = nc.declare_dram_parameter("ones", [1, P], F32, isOutput=False)
    vones_d = nc.declare_dram_parameter("vones", [P, NT, HL], F32, isOutput=False)
    if causal:
        m01 = nc.declare_dram_parameter("m01", [P, GL, SQW], F32, isOutput=False)
    else:
        m01 = nc.declare_dram_parameter("m01", [NT, P, NSQ, SQW], F32, isOutput=False)
    brows = {}
    for nm, use in (("bq", has_bq), ("bk", has_bk), ("bv", has_bv)):
        if use:
            brows[nm] = nc.declare_dram_parameter(nm + "r", [1, DL], F32, isOutput=False)
    out_d = nc.declare_dram_parameter("out", [S, D], F32, isOutput=True)

    with tile.TileContext(nc) as tc:
        with (
            tc.tile_pool(name="persist", bufs=1) as persist,
            tc.tile_pool(name="const", bufs=1) as cpool,
        ):
            qt_all = persist.tile([P, GL, S], F32R)   # Q^T (2 heads per group)
            kt_all = persist.tile([P, GL, S], F32R)   # K^T
            vaug = persist.tile([P, NT, HL, HD + 1], F32R)  # V with ones col

            ident = cpool.tile([P, P], F32R)
            nc.sync.dma_start(ident[:], _r(ident_d[:]))
            ones1 = cpool.tile([1, P], F32R)
            nc.sync.dma_start(ones1[:], _r(ones_d[:]))
            nc.sync.dma_start(
                vaug[:, :, :, HD : HD + 1], _r(vones_d[:].unsqueeze(3))
            )
            if causal:
                m01_sb = cpool.tile([P, GL, SQW], F32R)
                nc.sync.dma_start(m01_sb[:], _r(m01[:]))
            brow_sb = {}
            for nm, ap in brows.items():
                t = cpool.tile([1, DL], F32R)
                nc.sync.dma_start(t[:], _r(ap[:]))
                brow_sb[nm] = t

            # ---------------- Phase A: QKV projections + RoPE ----------------
            with (
                tc.tile_pool(name="wA", bufs=1) as wpool,
                tc.tile_pool(name="xA", bufs=3) as xpool,
                tc.tile_pool(name="cA", bufs=3) as cspool,
                tc.tile_pool(name="tA", bufs=3) as tpool,
                tc.tile_pool(name="psA", bufs=3, space="PSUM") as psA,
                tc.tile_pool(name="psT", bufs=2, space="PSUM") as psT,
            ):
                w_sb = {}
                for nm, ap in (("q", wq), ("k", wk), ("v", wv)):
                    t = wpool.tile([P, KCH, DL], F32R, tag="w" + nm)
                    nc.sync.dma_start(t[:], _r(ap[:]))
                    w_sb[nm] = t

                for t in range(NT):
                    x_sb = xpool.tile([P, KCH, P], F32R, tag="x")
                    nc.sync.dma_start(x_sb[:], _r(xtp[t]))
                    cos_sb = cspool.tile([P, HD], F32, tag="cos")
                    nc.sync.dma_start(cos_sb[:], cos_d[t])
                    ss_sb = cspool.tile([P, HD], F32, tag="ss")
                    nc.sync.dma_start(ss_sb[:], ss_d[t])

                    for nm in ("q", "k", "v"):
                        ps = psA.tile([P, DL], F32, tag="proj")
                        use_b = (nm == "q" and has_bq) or (nm == "k" and has_bk) or (
                            nm == "v" and has_bv
                        )
                        if use_b:
                            nc.tensor.matmul(
                                ps[:], ones1[0:1, :], brow_sb["b" + nm][:],
                                start=True, stop=False,
                            )
                        for kc in range(KCH):
                            nc.tensor.matmul(
                                ps[:], x_sb[:, kc, :], w_sb[nm][:, kc, :],
                                start=(kc == 0 and not use_b), stop=(kc == KCH - 1),
                            )
                        if nm == "v":
                            nc.vector.tensor_copy(
                                vaug[:, t, :, 0:HD],
                                ps[:].rearrange("p (h e) -> p h e", h=HL),
                            )
                            continue
                        # RoPE: term_c = q*cos ; term_s = swap_halves(q)*ss
                        ps_h = ps[:].rearrange("p (h e) -> p h e", h=HL)
                        ps_r = ps[:].rearrange("p (h t e) -> p h t e", h=HL, t=2)
                        term_c = tpool.tile([P, DL], F32R, tag="tc")
                        nc.vector.tensor_tensor(
                            out=term_c[:].rearrange("p (h e) -> p h e", h=HL),
                            in0=ps_h,
                            in1=cos_sb[:].unsqueeze(1).broadcast_to([P, HL, HD]),
                            op=mybir.AluOpType.mult,
                        )
                        term_s = tpool.tile([P, DL], F32R, tag="ts")
                        nc.vector.tensor_tensor(
                            out=term_s[:].rearrange("p (h t e) -> p h t e", h=HL, t=2),
                            in0=ps_r[:, :, ::-1, :],
                            in1=ss_sb[:]
                            .rearrange("p (t e) -> p t e", t=2)
                            .unsqueeze(1)
                            .broadcast_to([P, HL, 2, 32]),
                            op=mybir.AluOpType.mult,
                        )
                        # transpose-accumulate the two terms into Q^T/K^T psum
                        trp = psT.tile([P, NSQ, P], F32, tag="tr")
                        for j in range(GL):
                            nc.tensor.matmul(
                                _r(trp[:, j, :]), term_c[:, j * P : (j + 1) * P],
                                ident[:], is_transpose=True, start=True, stop=False,
                            )
                            nc.tensor.matmul(
                                _r(trp[:, j, :]), term_s[:, j * P : (j + 1) * P],
                                ident[:], is_transpose=True, start=False, stop=True,
                            )
                        dest = qt_all if nm == "q" else kt_all
                        nc.vector.tensor_copy(dest[:, :, t * P : (t + 1) * P], trp[:])

            # ---------------- Phase B: attention + output projection ---------
            with (
                tc.tile_pool(name="persistB", bufs=1) as persistB,
                tc.tile_pool(name="wo", bufs=1) as wopool,
                tc.tile_pool(name="mg", bufs=3) as mgpool,
                tc.tile_pool(name="pp", bufs=6) as ppool,
                tc.tile_pool(name="rp", bufs=2) as rpool,
                tc.tile_pool(name="op", bufs=3) as opool,
                tc.tile_pool(name="psS", bufs=3, space="PSUM") as psS,
                tc.tile_pool(name="psC", bufs=2, space="PSUM") as psC,
                tc.tile_pool(name="psR", bufs=2, space="PSUM") as psR,
                tc.tile_pool(name="psO", bufs=1, space="PSUM") as psO,
            ):
                ctxn = persistB.tile([P, GL, S], F32R)  # normalized ctx^T
                wo_sb = wopool.tile([P, GL, D], F32R)
                nc.sync.dma_start(wo_sb[:], _r(wo[:]))

                for c in range(NSQ):
                    sq = slice(c * SQW, (c + 1) * SQW)
                    kmax = NSQ * (c + 1) if causal else NT
                    for j in range(GL):
                        ctxs = [
                            psC.tile([HD + 1, SQW], F32, name="ctx0", tag="ctx"),
                            psC.tile([HD + 1, SQW], F32, name="ctx1", tag="ctx"),
                        ]
                        for k in range(kmax):
                            pts = []
                            for e in range(2):
                                pr = slice(e * HD, (e + 1) * HD)
                                st = psS.tile([P, SQW], F32, tag="st")
                                nc.tensor.matmul(
                                    st[:],
                                    kt_all[pr, j, k * P : (k + 1) * P],
                                    qt_all[pr, j, sq],
                                    start=True, stop=True,
                                    tile_position=(e * HD, 0),
                                )
                                pt = ppool.tile([P, SQW], F32R, tag="pt")
                                nc.scalar.activation(
                                    pt[:], st[:], mybir.ActivationFunctionType.Exp
                                )
                                pts.append(pt)
                            if causal:
                                if k >= NSQ * c:
                                    u = k - NSQ * c
                                    fd = P * (u + 1)
                                    for pt in pts:
                                        nc.vector.tensor_tensor(
                                            out=pt[:, 0:fd], in0=pt[:, 0:fd],
                                            in1=m01_sb[:, u, 0:fd],
                                            op=mybir.AluOpType.mult,
                                        )
                            else:
                                mg = mgpool.tile([P, SQW], F32R, tag="mg")
                                nc.sync.dma_start(mg[:], _r(m01[k, :, c, :]))
                                for pt in pts:
                                    nc.vector.tensor_tensor(
                                        out=pt[:], in0=pt[:], in1=mg[:],
                                        op=mybir.AluOpType.mult,
                                    )
                            for e in range(2):
                                nc.tensor.matmul(
                                    ctxs[e][:],
                                    vaug[:, k, 2 * j + e, :],
                                    pts[e][:],
                                    start=(k == 0), stop=(k == kmax - 1),
                                )
                        for e in range(2):
                            srow = rpool.tile([1, SQW], F32R, tag="srow")
                            nc.vector.tensor_copy(srow[:], ctxs[e][HD : HD + 1, :])
                            rps = psR.tile([HD, SQW], F32, tag="rps")
                            nc.tensor.matmul(
                                rps[:], ones1[0:1, 0:HD], srow[:],
                                start=True, stop=True,
                            )
                            rr = rpool.tile([HD, SQW], F32, tag="rr")
                            nc.vector.reciprocal_approx_fast(rr[:], rps[:])
                            nc.vector.tensor_tensor(
                                out=ctxn[e * HD : (e + 1) * HD, j, sq],
                                in0=ctxs[e][0:HD, :], in1=rr[:],
                                op=mybir.AluOpType.mult,
                            )
                    # output projection for this sq chunk
                    for tt in range(c * NSQ, (c + 1) * NSQ):
                        for n in range(2):
                            pso = psO.tile([P, SQW], F32, tag="pso")
                            for dl in range(GL):
                                nc.tensor.matmul(
                                    pso[:],
                                    ctxn[:, dl, tt * P : (tt + 1) * P],
                                    wo_sb[:, dl, n * SQW : (n + 1) * SQW],
                                    start=(dl == 0), stop=(dl == GL - 1),
                                )
                            osb = opool.tile([P, SQW], F32, tag="osb")
                            nc.scalar.copy(osb[:], pso[:])
                            nc.sync.dma_start(
                                out_d[tt * P : (tt + 1) * P, n * SQW : (n + 1) * SQW],
                                osb[:],
                            )
    nc.compile()
    return nc


_NC_CACHE = {}


def kernel(x, mask, rope_cos, rope_sin, Wq, bq, Wk, bk, Wv, bv, Wo, bo):
    x = np.asarray(x, np.float32)
    mask = np.asarray(mask, bool)
    rope_cos = np.asarray(rope_cos, np.float32)
    rope_sin = np.asarray(rope_sin, np.float32)
    Wq, Wk, Wv, Wo = (np.asarray(w, np.float32) for w in (Wq, Wk, Wv, Wo))
    bq, bk, bv, bo = (np.asarray(b_, np.float32) for b_ in (bq, bk, bv, bo))

    causal = bool(np.array_equal(mask, np.triu(np.ones((S, S), bool), k=1)))
    has_bq, has_bk, has_bv = bool(bq.any()), bool(bk.any()), bool(bv.any())

    key = (causal, has_bq, has_bk, has_bv)
    if key not in _NC_CACHE:
        _NC_CACHE[key] = _build(*key)
    nc = _NC_CACHE[key]

    scale = np.float32(1.0 / np.sqrt(HD))
    cosp = np.ascontiguousarray(rope_cos.reshape(NT, P, HD))
    ssarr = rope_sin.copy()
    ssarr[:, 0:32] = -ssarr[:, 0:32]
    ssp = np.ascontiguousarray(ssarr.reshape(NT, P, HD))
    if causal:
        pp, uu, qq = np.meshgrid(
            np.arange(P), np.arange(GL), np.arange(SQW), indexing="ij"
        )
        m01 = ((P * uu + pp) <= qq).astype(np.float32)  # [P, GL, SQW]
    else:
        m01 = np.ascontiguousarray(
            (~mask).astype(np.float32).T.reshape(NT, P, NSQ, SQW)
        )

    in_maps = []
    xtp_b = {}
    for core in range(8):
        b_, g = core // 2, core % 2
        if b_ not in xtp_b:
            xtp_b[b_] = np.ascontiguousarray(
                x[b_].reshape(NT, P, KCH, P).transpose(0, 3, 2, 1)
            )
        gs = g * DL
        wq_p = np.ascontiguousarray(
            (Wq[gs : gs + DL, :] * scale).T.reshape(KCH, P, DL).transpose(1, 0, 2)
        )
        wk_p = np.ascontiguousarray(
            Wk[gs : gs + DL, :].T.reshape(KCH, P, DL).transpose(1, 0, 2)
        )
        wv_p = np.ascontiguousarray(
            Wv[gs : gs + DL, :].T.reshape(KCH, P, DL).transpose(1, 0, 2)
        )
        wo_p = np.ascontiguousarray(
            Wo[:, gs : gs + DL].T.reshape(GL, P, D).transpose(1, 0, 2)
        )
        m = {
            "xtp": xtp_b[b_], "wq": wq_p, "wk": wk_p, "wv": wv_p, "wo": wo_p,
            "cos": cosp, "ss": ssp, "m01": m01,
            "ident": np.eye(P, dtype=np.float32),
            "ones": np.ones((1, P), np.float32),
            "vones": np.ones((P, NT, HL), np.float32),
        }
        if has_bq:
            m["bqr"] = (bq[gs : gs + DL] * scale)[None, :].astype(np.float32)
        if has_bk:
            m["bkr"] = bk[gs : gs + DL][None, :].astype(np.float32)
        if has_bv:
            m["bvr"] = bv[gs : gs + DL][None, :].astype(np.float32)
        in_maps.append(m)

    res = run_bass_kernel_spmd(nc, in_maps, list(range(8)))
    out = np.empty((B, S, D), np.float32)
    for b_ in range(B):
        out[b_] = res.results[2 * b_]["out"] + res.results[2 * b_ + 1]["out"]
    out += bo[None, None, :]
    return out


# revision 15
# speedup vs baseline: 1.1430x; 1.1430x over previous
"""Trainium2 Bass kernel for CustomMHA (B=4, S=2048, D=1024, H=16, rope, causal).

Sharding: 8 cores = 4 batches x 2 head-groups (8 heads each).
Per core, per sq-chunk of 512 (interleaved so PE-bound projection work overlaps
ACT-bound softmax work): QKV projections in f32r, RoPE via DVE muls + PE
transpose-accumulate (bf16), attention as S^T = K^T.T @ Q^T head-pair tiles
(single merged exp on ACT -> P^T in SBUF bf16, no P transpose), AV with a
ones-row appended to V so softmax denominators fall out of the same matmul,
deferred per-head normalization (PE partition-broadcast + DVE fast reciprocal),
then the output projection of the local 512 ctx dims. Host sums the two
head-group partials per batch.
"""

import numpy as np

import concourse.bass as bass
import concourse.tile as tile
from concourse import bacc, mybir
from concourse.bass_utils import run_bass_kernel_spmd

B, S, D, H = 4, 2048, 1024, 16
HD = 64          # head dim
P = 128          # partitions
NT = S // P      # 16 s-tiles
HL = H // 2      # 8 heads per core
GL = HL // 2     # 4 head-pair groups per core
DL = HL * HD     # 512 local ctx dims
KCH = D // P     # 8 contraction chunks
NSQ = 4          # sq chunks
SQW = S // NSQ   # 512
F32 = mybir.dt.float32
F32R = mybir.dt.float32r
BF16 = mybir.dt.bfloat16
EXP = mybir.ActivationFunctionType.Exp
MUL = mybir.AluOpType.mult


def _r(ap):
    return ap.bitcast(F32R)


def _build(causal: bool, has_bq: bool, has_bk: bool, has_bv: bool):
    nc = bacc.Bacc("TRN2", target_bir_lowering=False, debug=False, num_devices=8)

    xtp = nc.declare_dram_parameter("xtp", [NT, P, KCH, P], F32, isOutput=False)
    wq = nc.declare_dram_parameter("wq", [P, KCH, DL], F32, isOutput=False)
    wk = nc.declare_dram_parameter("wk", [P, KCH, DL], F32, isOutput=False)
    wv = nc.declare_dram_parameter("wv", [P, KCH, DL], F32, isOutput=False)
    wo = nc.declare_dram_parameter("wo", [P, GL, D], BF16, isOutput=False)
    cos_d = nc.declare_dram_parameter("cos", [NT, P, HD], F32, isOutput=False)
    ss_d = nc.declare_dram_parameter("ss", [NT, P, HD], F32, isOutput=False)
    ident_d = nc.declare_dram_parameter("ident", [P, P], BF16, isOutput=False)
    ones_d = nc.declare_dram_parameter("ones", [1, P], F32, isOutput=False)
    vones_d = nc.declare_dram_parameter("vones", [P, NT, HL], BF16, isOutput=False)
    if causal:
        m01 = nc.declare_dram_parameter("m01", [P, 2 * P], BF16, isOutput=False)
    else:
        m01 = nc.declare_dram_parameter("m01", [NT, P, NSQ, SQW], BF16,
                                        isOutput=False)
    brows = {}
    for nm, use in (("bq", has_bq), ("bk", has_bk), ("bv", has_bv)):
        if use:
            brows[nm] = nc.declare_dram_parameter(nm + "r", [1, DL], F32,
                                                  isOutput=False)
    out_d = nc.declare_dram_parameter("out", [S, D], F32, isOutput=True)

    with tile.TileContext(nc) as tc:
        with (
            tc.tile_pool(name="persist", bufs=1) as persist,
            tc.tile_pool(name="const", bufs=1) as cpool,
            tc.tile_pool(name="wA", bufs=1) as wpool,
            tc.tile_pool(name="xA", bufs=3) as xpool,
            tc.tile_pool(name="cA", bufs=3) as cspool,
            tc.tile_pool(name="tA", bufs=3) as tpool,
            tc.tile_pool(name="wo", bufs=1) as wopool,
            tc.tile_pool(name="mg", bufs=3) as mgpool,
            tc.tile_pool(name="pp", bufs=6) as ppool,
            tc.tile_pool(name="np", bufs=4) as npool,
            tc.tile_pool(name="rp", bufs=2) as rpool,
            tc.tile_pool(name="op", bufs=3) as opool,
            tc.tile_pool(name="psA", bufs=2, space="PSUM") as psA,
            tc.tile_pool(name="psT", bufs=1, space="PSUM") as psT,
            tc.tile_pool(name="psS", bufs=3, space="PSUM") as psS,
            tc.tile_pool(name="psC", bufs=2, space="PSUM") as psC,
        ):
            qt_all = persist.tile([P, GL, S], BF16)   # Q^T (2 heads per group)
            kt_all = persist.tile([P, GL, S], BF16)   # K^T
            vaug = persist.tile([P, NT, HL, HD + 1], BF16)  # V with ones col
            ctxn = persist.tile([P, GL, S], BF16)     # normalized ctx^T

            ident = cpool.tile([P, P], BF16)
            ones1 = cpool.tile([1, P], F32R)
            if causal:
                m01_sb = cpool.tile([P, 2 * P], BF16)
            brow_sb = {nm: cpool.tile([1, DL], F32R, name="brow_" + nm)
                       for nm in brows}

            first = {}
            first["x"] = xpool.tile([P, KCH, P], F32R, tag="x", name="x0")
            first["cos"] = cspool.tile([P, HD], F32, tag="cos", name="cos0")
            first["ss"] = cspool.tile([P, HD], F32, tag="ss", name="ss0")
            w_sb = {}
            for nm in ("q", "k", "v"):
                w_sb[nm] = wpool.tile([P, KCH, DL], F32R, tag="w" + nm,
                                      name="w" + nm)
            # interleave first x-tile chunks with weight chunks so the first
            # projection matmul starts after ~2 small DMAs
            for kc in range(KCH):
                nc.sync.dma_start(first["x"][:, kc, :], _r(xtp[0, :, kc, :]))
                for nm, ap in (("q", wq), ("k", wk), ("v", wv)):
                    nc.sync.dma_start(w_sb[nm][:, kc, :], _r(ap[:, kc, :]))
            nc.sync.dma_start(first["cos"][:], cos_d[0])
            nc.sync.dma_start(first["ss"][:], ss_d[0])
            nc.sync.dma_start(ident[:], ident_d[:])
            nc.sync.dma_start(ones1[:], _r(ones_d[:]))
            nc.sync.dma_start(vaug[:, :, :, HD : HD + 1], vones_d[:].unsqueeze(3))
            if causal:
                nc.sync.dma_start(m01_sb[:], m01[:])
            for nm, ap in brows.items():
                nc.sync.dma_start(brow_sb[nm][:], _r(ap[:]))
            wo_sb = wopool.tile([P, GL, D], BF16)
            nc.sync.dma_start(wo_sb[:], wo[:])

            def phase_a(t):
                """Project x-tile t to Q/K/V, rope Q/K, transpose into qt/kt."""
                if t == 0:
                    x_sb, cos_sb, ss_sb = first["x"], first["cos"], first["ss"]
                else:
                    x_sb = xpool.tile([P, KCH, P], F32R, tag="x", name="x")
                    nc.sync.dma_start(x_sb[:], _r(xtp[t]))
                    cos_sb = cspool.tile([P, HD], F32, tag="cos", name="cos")
                    nc.sync.dma_start(cos_sb[:], cos_d[t])
                    ss_sb = cspool.tile([P, HD], F32, tag="ss", name="ss")
                    nc.sync.dma_start(ss_sb[:], ss_d[t])

                trp = psT.tile([P, 2, NSQ, P], BF16, name="trp")
                for nm in ("q", "k", "v"):
                    ps = psA.tile([P, DL], F32, tag="proj", name="proj")
                    use_b = (nm == "q" and has_bq) or (nm == "k" and has_bk) or (
                        nm == "v" and has_bv
                    )
                    if use_b:
                        nc.tensor.matmul(
                            ps[:], ones1[0:1, :], brow_sb["b" + nm][:],
                            start=True, stop=False,
                        )
                    for kc in range(KCH):
                        nc.tensor.matmul(
                            ps[:], x_sb[:, kc, :], w_sb[nm][:, kc, :],
                            start=(kc == 0 and not use_b), stop=(kc == KCH - 1),
                        )
                    if nm == "v":
                        nc.scalar.copy(
                            vaug[:, t, :, 0:HD],
                            ps[:].rearrange("p (h e) -> p h e", h=HL),
                        )
                        continue
                    # RoPE: term_c = q*cos ; term_s = swap_halves(q)*ss
                    ps_h = ps[:].rearrange("p (h e) -> p h e", h=HL)
                    ps_r = ps[:].rearrange("p (h t e) -> p h t e", h=HL, t=2)
                    term_c = tpool.tile([P, DL], BF16, tag="tc", name="tc")
                    nc.vector.tensor_tensor(
                        out=term_c[:].rearrange("p (h e) -> p h e", h=HL),
                        in0=ps_h,
                        in1=cos_sb[:].unsqueeze(1).broadcast_to([P, HL, HD]),
                        op=MUL,
                    )
                    term_s = tpool.tile([P, DL], BF16, tag="ts", name="ts")
                    nc.vector.tensor_tensor(
                        out=term_s[:].rearrange("p (h t e) -> p h t e", h=HL, t=2),
                        in0=ps_r[:, :, ::-1, :],
                        in1=ss_sb[:]
                        .rearrange("p (t e) -> p t e", t=2)
                        .unsqueeze(1)
                        .broadcast_to([P, HL, 2, 32]),
                        op=MUL,
                    )
                    term = tpool.tile([P, DL], BF16, tag="tsum", name="tsum")
                    nc.vector.tensor_tensor(
                        out=term[:], in0=term_c[:], in1=term_s[:],
                        op=mybir.AluOpType.add,
                    )
                    qk = 0 if nm == "q" else 1
                    for j in range(GL):
                        nc.tensor.matmul(
                            trp[:, qk, j, :].bitcast(BF16),
                            term[:, j * P : (j + 1) * P],
                            ident[:], is_transpose=True, start=True, stop=True,
                        )
                nc.scalar.copy(qt_all[:, :, t * P : (t + 1) * P], trp[:, 0])
                nc.scalar.copy(kt_all[:, :, t * P : (t + 1) * P], trp[:, 1])

            def attention(c):
                sq = slice(c * SQW, (c + 1) * SQW)
                kmax = NSQ * (c + 1) if causal else NT
                for j in range(GL):
                    ctxs = [
                        psC.tile([HD + 1, SQW], F32, name="ctx0", tag="ctx"),
                        psC.tile([HD + 1, SQW], F32, name="ctx1", tag="ctx"),
                    ]
                    for k in range(kmax):
                        u = k - NSQ * c if causal else -1
                        off = min(u * P, SQW - 2 * P) if u >= 0 else 0
                        sqo = slice(c * SQW + off, (c + 1) * SQW)
                        pt = ppool.tile([P, 2 * SQW], BF16, tag="pt", name="pt")
                        pt3 = pt[:].rearrange("p (e q) -> p e q", e=2)
                        for e in range(2):
                            pr = slice(e * HD, (e + 1) * HD)
                            st = psS.tile([P, SQW], F32, tag="st", name="st")
                            nc.tensor.matmul(
                                st[:, off:],
                                kt_all[pr, j, k * P : (k + 1) * P],
                                qt_all[pr, j, sqo],
                                start=True, stop=True,
                                tile_position=(e * HD, 0),
                            )
                            nc.scalar.activation(
                                pt[:, e * SQW + off : (e + 1) * SQW],
                                st[:, off:], EXP,
                            )
                        if u >= 0:
                            if u < 3:
                                ms = slice(u * P, (u + 1) * P)
                                mi = m01_sb[:, P : 2 * P]
                            else:
                                ms = slice(2 * P, 4 * P)
                                mi = m01_sb[:]
                            mw = ms.stop - ms.start
                            nc.vector.tensor_tensor(
                                out=pt3[:, :, ms], in0=pt3[:, :, ms],
                                in1=mi.unsqueeze(1).broadcast_to([P, 2, mw]),
                                op=MUL,
                            )
                        elif not causal:
                            mg = mgpool.tile([P, SQW], BF16, tag="mg", name="mg")
                            nc.sync.dma_start(mg[:], m01[k, :, c, :])
                            nc.vector.tensor_tensor(
                                out=pt3[:], in0=pt3[:],
                                in1=mg[:].unsqueeze(1).broadcast_to([P, 2, SQW]),
                                op=MUL,
                            )
                        for e in range(2):
                            nc.tensor.matmul(
                                ctxs[e][:, off:],
                                vaug[:, k, 2 * j + e, :],
                                pt[:, e * SQW + off : (e + 1) * SQW],
                                start=(k == 0), stop=(k == kmax - 1),
                            )
                    for e in range(2):
                        # evict ctx+sums to SBUF fast to free the PSUM bank;
                        # the rest of the norm runs off the critical path
                        ctxe = npool.tile([HD + 1, SQW], F32R, tag="ctxe",
                                          name="ctxe")
                        nc.vector.tensor_copy(ctxe[:], ctxs[e][:])
                        srow = rpool.tile([1, SQW], F32R, tag="srow", name="srow")
                        nc.vector.tensor_copy(srow[:], ctxe[HD : HD + 1, :])
                        rps = psS.tile([HD, SQW], F32, tag="st", name="rps")
                        nc.tensor.matmul(
                            rps[:], ones1[0:1, 0:HD], srow[:],
                            start=True, stop=True,
                        )
                        rr = rpool.tile([HD, SQW], F32, tag="rr", name="rr")
                        nc.vector.reciprocal_approx_fast(rr[:], rps[:])
                        nc.vector.tensor_tensor(
                            out=ctxn[e * HD : (e + 1) * HD, j, sq],
                            in0=ctxe[0:HD, :], in1=rr[:],
                            op=MUL,
                        )

            def outproj(c):
                for tt in range(c * NSQ, (c + 1) * NSQ):
                    for n in range(2):
                        pso = psC.tile([P, SQW], F32, tag="ctx", name="pso")
                        for dl in range(GL):
                            nc.tensor.matmul(
                                pso[:],
                                ctxn[:, dl, tt * P : (tt + 1) * P],
                                wo_sb[:, dl, n * SQW : (n + 1) * SQW],
                                start=(dl == 0), stop=(dl == GL - 1),
                            )
                        osb = opool.tile([P, SQW], F32, tag="osb", name="osb")
                        nc.scalar.copy(osb[:], pso[:])
                        nc.sync.dma_start(
                            out_d[tt * P : (tt + 1) * P, n * SQW : (n + 1) * SQW],
                            osb[:],
                        )

            if causal:
                # interleave: chunk c's attention only needs s-tiles <= 4c+3
                for c in range(NSQ):
                    for t in range(c * NSQ, (c + 1) * NSQ):
                        phase_a(t)
                    attention(c)
                    outproj(c)
            else:
                for t in range(NT):
                    phase_a(t)
                for c in range(NSQ):
                    attention(c)
                    outproj(c)
    nc.compile()
    return nc


_NC_CACHE = {}


def kernel(x, mask, rope_cos, rope_sin, Wq, bq, Wk, bk, Wv, bv, Wo, bo):
    import ml_dtypes

    bf16 = ml_dtypes.bfloat16
    x = np.asarray(x, np.float32)
    mask = np.asarray(mask, bool)
    rope_cos = np.asarray(rope_cos, np.float32)
    rope_sin = np.asarray(rope_sin, np.float32)
    Wq, Wk, Wv, Wo = (np.asarray(w, np.float32) for w in (Wq, Wk, Wv, Wo))
    bq, bk, bv, bo = (np.asarray(b_, np.float32) for b_ in (bq, bk, bv, bo))

    causal = bool(np.array_equal(mask, np.triu(np.ones((S, S), bool), k=1)))
    has_bq, has_bk, has_bv = bool(bq.any()), bool(bk.any()), bool(bv.any())

    key = (causal, has_bq, has_bk, has_bv)
    if key not in _NC_CACHE:
        _NC_CACHE[key] = _build(*key)
    nc = _NC_CACHE[key]

    scale = np.float32(1.0 / np.sqrt(HD))
    cosp = np.ascontiguousarray(rope_cos.reshape(NT, P, HD))
    ssarr = rope_sin.copy()
    ssarr[:, 0:32] = -ssarr[:, 0:32]
    ssp = np.ascontiguousarray(ssarr.reshape(NT, P, HD))
    if causal:
        tri = np.arange(P)[:, None] <= np.arange(P)[None, :]
        m01 = np.concatenate([np.zeros((P, P), bf16), tri.astype(bf16)], axis=1)
    else:
        m01 = np.ascontiguousarray(
            (~mask).astype(bf16).T.reshape(NT, P, NSQ, SQW)
        )

    in_maps = []
    xtp_b = {}
    for core in range(8):
        b_, g = core // 2, core % 2
        if b_ not in xtp_b:
            xtp_b[b_] = np.ascontiguousarray(
                x[b_].reshape(NT, P, KCH, P).transpose(0, 3, 2, 1)
            )
        gs = g * DL
        wq_p = np.ascontiguousarray(
            (Wq[gs : gs + DL, :] * scale).T.reshape(KCH, P, DL).transpose(1, 0, 2)
        )
        wk_p = np.ascontiguousarray(
            Wk[gs : gs + DL, :].T.reshape(KCH, P, DL).transpose(1, 0, 2)
        )
        wv_p = np.ascontiguousarray(
            Wv[gs : gs + DL, :].T.reshape(KCH, P, DL).transpose(1, 0, 2)
        )
        wo_p = np.ascontiguousarray(
            Wo[:, gs : gs + DL].T.reshape(GL, P, D).transpose(1, 0, 2)
        ).astype(bf16)
        m = {
            "xtp": xtp_b[b_], "wq": wq_p, "wk": wk_p, "wv": wv_p, "wo": wo_p,
            "cos": cosp, "ss": ssp, "m01": m01,
            "ident": np.eye(P, dtype=bf16),
            "ones": np.ones((1, P), np.float32),
            "vones": np.ones((P, NT, HL), bf16),
        }
        if has_bq:
            m["bqr"] = (bq[gs : gs + DL] * scale)[None, :].astype(np.float32)
        if has_bk:
            m["bkr"] = bk[gs : gs + DL][None, :].astype(np.float32)
        if has_bv:
            m["bvr"] = bv[gs : gs + DL][None, :].astype(np.float32)
        in_maps.append(m)

    res = run_bass_kernel_spmd(nc, in_maps, list(range(8)))
    out = np.empty((B, S, D), np.float32)
    for b_ in range(B):
        out[b_] = res.results[2 * b_]["out"] + res.results[2 * b_ + 1]["out"]
    out += bo[None, None, :]
    return out
